# revision 1
# baseline (speedup 1.0000x reference)
"""Block-tridiagonal iterative MLP on 8 TRN2 NeuronCores.

Strategy: tensor-parallel split of every W block along the output-feature dim
(256 features per core). Activations are kept transposed [feature, batch] so
each iteration's output feeds the next matmul directly. Between the two
iterations each core's [4, 256, 512] activation slice is AllGathered per
block (4 collectives, overlapped with compute). Matmuls run in bf16 with
fp32 PSUM accumulation; the per-row bias is folded into the PSUM group as a
K=1 rank-1 matmul against a ones vector.
"""
import sys

sys.path.insert(0, "/opt/trn_rl_repo")

import numpy as np
import ml_dtypes

import concourse.bass as bass
import concourse.mybir as mybir
from concourse.bass_utils import run_bass_kernel_spmd

N_CORES = 8
NUM_BLOCKS = 4
BLOCK_SIZE = 2048
BATCH = 512
BLOCK_PAIRS = [(0, 0), (0, 1), (1, 0), (1, 1), (1, 2),
               (2, 1), (2, 2), (2, 3), (3, 2), (3, 3)]
ROWS = {i: [(k, j) for k, (ii, j) in enumerate(BLOCK_PAIRS) if ii == i]
        for i in range(NUM_BLOCKS)}

P = 128
OSL = BLOCK_SIZE // N_CORES          # 256 out features per core
NOT = OSL // P                       # 2 output tiles per block per core
NET = BLOCK_SIZE // P                # 16 contraction tiles
BF = mybir.dt.bfloat16
F32 = mybir.dt.float32


def build_nc(mock_cc=False):
    nc = bass.Bass(num_devices=N_CORES)

    wt = nc.dram_tensor("wt", [10, P, NET, OSL], BF, kind="ExternalInput")
    a0 = nc.dram_tensor("a0", [NUM_BLOCKS, P, NET, BATCH], BF, kind="ExternalInput")
    biasw = nc.dram_tensor("biasw", [1, NUM_BLOCKS * OSL], BF, kind="ExternalInput")
    ones = nc.dram_tensor("ones", [1, BATCH], BF, kind="ExternalInput")
    y_out = nc.dram_tensor("y", [NUM_BLOCKS, NOT, P, BATCH], F32, kind="ExternalOutput")

    cc_in = nc.dram_tensor("cc_in", [NUM_BLOCKS, NOT, P, BATCH], BF)
    cc_out = nc.dram_tensor("cc_out", [NUM_BLOCKS, BLOCK_SIZE, BATCH], BF,
                            addr_space="Shared")

    with (
        nc.sbuf_tensor("wt_sb", [P, 10 * NET * OSL], BF) as wt_sb_flat,
        nc.sbuf_tensor("a_sb", [P, NUM_BLOCKS * NET * BATCH], BF) as a_sb_flat,
        nc.sbuf_tensor("bias_sb", [1, NUM_BLOCKS * OSL], BF) as bias_sb,
        nc.sbuf_tensor("ones_sb", [1, BATCH], BF) as ones_sb,
        nc.sbuf_tensor("act_sb", [P, 8 * BATCH], BF) as act_sb_flat,
        nc.sbuf_tensor("yf_sb", [P, 8 * BATCH], F32) as yf_sb_flat,
        nc.psum_tensor("ps", [P, 8 * BATCH], F32) as ps_flat,
        nc.Block() as block,
    ):
        import contextlib
        _sem_stack = contextlib.ExitStack()
        wt_sems = [_sem_stack.enter_context(nc.semaphore(f"wt_sem{k}")) for k in range(10)]
        a0_sems = [_sem_stack.enter_context(nc.semaphore(f"a0_sem{j}")) for j in range(4)]
        a1_sems = [_sem_stack.enter_context(nc.semaphore(f"a1_sem{j}")) for j in range(4)]
        cin_sems = [_sem_stack.enter_context(nc.semaphore(f"cin_sem{i}")) for i in range(4)]
        misc_sem = _sem_stack.enter_context(nc.semaphore("misc_sem"))
        cc_sem = _sem_stack.enter_context(nc.semaphore("cc_sem"))
        pe_sem = _sem_stack.enter_context(nc.semaphore("pe_sem"))
        act_sem = _sem_stack.enter_context(nc.semaphore("act_sem"))
        out_sem = _sem_stack.enter_context(nc.semaphore("out_sem"))
        def wt_ap(k, et, ot):        # lhsT [128(e), 128(o)]
            base = (k * NET + et) * OSL + ot * P
            return wt_sb_flat[:, base:base + P]

        def a_ap(j, et):             # rhs [128(e), 512(b)]
            base = (j * NET + et) * BATCH
            return a_sb_flat[:, base:base + BATCH]

        def ps_ap(g):                # psum group g in 0..7 -> one bank
            return ps_flat[:, g * BATCH:(g + 1) * BATCH]

        def act_ap(g):
            return act_sb_flat[:, g * BATCH:(g + 1) * BATCH]

        def yf_ap(g):
            return yf_sb_flat[:, g * BATCH:(g + 1) * BATCH]

        def bias_ap(i, ot):          # lhsT [1, 128]
            base = i * OSL + ot * P
            return bias_sb[:, base:base + P]

        # last PE group index (cumulative) whose MMs read a-slot j in iter 1:
        # slot j is read by rows j-1, j, j+1 -> last group = row min(j+1,3), ot=1
        slot_war = {j: 2 * (min(j + 1, NUM_BLOCKS - 1) + 1) for j in range(NUM_BLOCKS)}

        @block.sync
        def _(sp: bass.BassEngine):
            sp.dma_start(ones_sb[:, :], ones[:, :]).then_inc(misc_sem, 16)
            sp.dma_start(bias_sb[:, :], biasw[:, :]).then_inc(misc_sem, 16)

            def load_wt(k):
                sp.dma_start(
                    wt_sb_flat[:, k * NET * OSL:(k + 1) * NET * OSL],
                    wt[k].rearrange("p et o -> p (et o)"),
                ).then_inc(wt_sems[k], 16)

            def load_a0(j):
                sp.dma_start(
                    a_sb_flat[:, j * NET * BATCH:(j + 1) * NET * BATCH],
                    a0[j].rearrange("p et b -> p (et b)"),
                ).then_inc(a0_sems[j], 16)

            # interleave so row-0 deps (wt0, wt1, a0_0, a0_1) land first
            load_wt(0); load_wt(1); load_a0(0); load_a0(1)
            load_wt(2); load_wt(3); load_wt(4); load_a0(2)
            load_wt(5); load_wt(6); load_wt(7); load_a0(3)
            load_wt(8); load_wt(9)
            # iter-1 activations -> cc_in bounce
            for g in range(8):
                i, ot = g // 2, g % 2
                sp.wait_ge(act_sem, g + 1)
                sp.dma_start(cc_in[i, ot], act_ap(g)).then_inc(cin_sems[i], 16)
            # iter-2 activation loads from gathered buffers
            for j in range(NUM_BLOCKS):
                sp.wait_ge(cc_sem, (16 if mock_cc else 1) * (j + 1))
                sp.wait_ge(pe_sem, slot_war[j])
                sp.dma_start(
                    a_sb_flat[:, j * NET * BATCH:(j + 1) * NET * BATCH]
                    .rearrange("p (et b) -> p et b", et=NET),
                    cc_out[j].rearrange("(et p) b -> p et b", p=P),
                ).then_inc(a1_sems[j], 16)
            # outputs
            for g in range(8):
                i, ot = g // 2, g % 2
                sp.wait_ge(act_sem, 8 + g + 1)
                sp.dma_start(y_out[i, ot], yf_ap(g)).then_inc(out_sem, 16)

        @block.gpsimd
        def _(gp: bass.BassGpSimd):
            for i in range(NUM_BLOCKS):
                gp.wait_ge(cin_sems[i], 32)
                if mock_cc:
                    # timing-sim stand-in: local copy of the same byte volume
                    gp.dma_start(
                        cc_out[i, 0:NOT * P],
                        cc_in[i].rearrange("t p b -> (t p) b"),
                    ).then_inc(cc_sem, 16)
                else:
                    gp.collective_compute(
                        "AllGather",
                        mybir.AluOpType.bypass,
                        replica_groups=[list(range(N_CORES))],
                        ins=[cc_in[i].opt()],
                        outs=[cc_out[i].opt()],
                    ).then_inc(cc_sem, 1)

        @block.tensor
        def _(pe: bass.BassTensorEngine):
            groups_done = 0
            for it in range(2):
                for i in range(NUM_BLOCKS):
                    pairs = ROWS[i]
                    for ot in range(NOT):
                        g = 2 * i + ot
                        if it == 0:
                            if g == 0:
                                pe.wait_ge(misc_sem, 32)
                            for k, j in pairs:
                                pe.wait_ge(wt_sems[k], 16)
                                pe.wait_ge(a0_sems[j], 16)
                        else:
                            for _, j in pairs:
                                pe.wait_ge(a1_sems[j], 16)
                            pe.wait_ge(act_sem, g + 1)  # PSUM bank WAR
                        first = True
                        for k, j in pairs:
                            for et in range(NET):
                                pe.matmul(ps_ap(g), wt_ap(k, et, ot), a_ap(j, et),
                                          start=first, stop=False)
                                first = False
                        groups_done += 1
                        pe.matmul(ps_ap(g), bias_ap(i, ot), ones_sb[:, :],
                                  start=False, stop=True).then_inc(pe_sem, 1)

        @block.scalar
        def _(ac: bass.BassScalarEngine):
            for it in range(2):
                for g in range(8):
                    n = it * 8 + g
                    ac.wait_ge(pe_sem, n + 1)
                    dst = act_ap(g) if it == 0 else yf_ap(g)
                    ac.activation(dst, ps_ap(g),
                                  mybir.ActivationFunctionType.Relu).then_inc(act_sem, 1)

    return nc


def _prep_inputs(X, W, b):
    """Host-side shard/layout prep (pure numpy, per-core views)."""
    bf = ml_dtypes.bfloat16
    # X^T tiles, shared by all cores: [4, 128(p), 16(et), 512(b)]
    a0 = np.ascontiguousarray(
        X.reshape(NUM_BLOCKS, BATCH, NET, P).transpose(0, 3, 2, 1)).astype(bf)
    ones = np.ones((1, BATCH), dtype=bf)
    # summed bias per out-block
    B = np.zeros((NUM_BLOCKS, BLOCK_SIZE), dtype=np.float32)
    for k, (i, _) in enumerate(BLOCK_PAIRS):
        B[i] += b[k]
    in_maps = []
    for c in range(N_CORES):
        Wc = W[:, c * OSL:(c + 1) * OSL, :]                       # [10, 256, 2048]
        wt = np.ascontiguousarray(
            Wc.reshape(10, OSL, NET, P).transpose(0, 3, 2, 1)).astype(bf)
        biasw = np.ascontiguousarray(
            B[:, c * OSL:(c + 1) * OSL].reshape(1, NUM_BLOCKS * OSL)).astype(bf)
        in_maps.append({"wt": wt, "a0": a0, "biasw": biasw, "ones": ones})
    return in_maps


_CACHE = {}


def kernel(X, W, b, _want_time=False):
    X = np.asarray(X, dtype=np.float32)
    W = np.asarray(W, dtype=np.float32)
    b = np.asarray(b, dtype=np.float32)
    in_maps = _prep_inputs(X, W, b)
    if "nc" not in _CACHE:
        _CACHE["nc"] = build_nc()
    try:
        res = run_bass_kernel_spmd(_CACHE["nc"], in_maps,
                                   core_ids=list(range(N_CORES)),
                                   trace=bool(_want_time))
    except ModuleNotFoundError:
        res = run_bass_kernel_spmd(_CACHE["nc"], in_maps,
                                   core_ids=list(range(N_CORES)))
    out = np.empty((NUM_BLOCKS, BATCH, BLOCK_SIZE), dtype=np.float32)
    for c in range(N_CORES):
        y = res.results[c]["y"]                                   # [4, 2, 128, 512]
        out[:, :, c * OSL:(c + 1) * OSL] = y.transpose(0, 3, 1, 2).reshape(
            NUM_BLOCKS, BATCH, OSL)
    if _want_time:
        return out, getattr(res, "exec_time_ns", None)
    return out

